# revision 1
# baseline (speedup 1.0000x reference)
"""Trainium2 Bass kernel for nn_CombinatorialClassifierSplit.

Reference computation:
    xr = x.reshape(B, P, S)
    logits = einsum('bps,pks', xr, W) + b          # (B, P, K)
    logp = log_softmax(logits, axis=2)
    out[b, c] = sum_p logp[b, p, idx[p, c]]        # (B, C)

Key restructuring: since idx doesn't depend on b,
    out[b, c] = sum_p logits[b, p, idx[p, c]] - LSE[b]
with LSE[b] = sum_p logsumexp_k(logits[b, p, :]).  The first term is a
plain matmul  M = x_flat @ Wg + bsum[c]  where Wg[(p,s), c] = W[p, idx[p,c], s]
and bsum[c] = sum_p b[p, idx[p,c]] are host-side gathers of the *static*
index tensor.  The device computes, per core (classes C sharded 8 ways):
  - per-p matmuls for logits -> exp (ACT) -> segmented sums (DVE); the raw
    per-(b,p) exp-sums ship as their own tiny fp32 output and the HOST
    finishes LSE[b] = sum_p ln(sums[b,p]) and applies `M - LSE`, keeping
    the whole softmax chain OFF the device's output critical path
  - the big matmul (contract 2048) in fp8 DoubleRowSwInterleave mode:
    the host-gathered Wg shard is the STATIONARY operand, pre-interleaved
    on the host into the dual-row fp8 layout the TRN2 PE requires
    (A/B column pairs, columns reversed); x^T is the moving operand and
    keeps its plain layout, shared with the logits path.  Output lands
    class-major ([class, batch]); the host transposes it back.
  - + bsum via rank-1 matmuls, psum->sbuf cast, bf16 DMA out.

All matmul operands are fp8e4 (e4m3): x is pre-scaled by 1/2 and W by 2
on the host (the scales cancel in x@W), which centers both operand
distributions inside e4m3's normal range.  M ~ N(0, 5.7) so bf16 output
rounding is ~0.03 versus an error budget of ~3.8.
"""

import numpy as np
import ml_dtypes

import concourse.bacc as bacc
import concourse.tile as tile
from concourse import mybir
from concourse.bass_utils import run_bass_kernel_spmd

F8 = ml_dtypes.float8_e4m3
BF16 = ml_dtypes.bfloat16

B, P, K, S, C = 128, 32, 100, 64, 10000
N_CORES = 8
CS = C // N_CORES          # 1250 classes per core
NT = (P * S) // 128        # 16 contract chunks of 128
NPAIR = NT // 2            # DoubleRow processes chunk pairs
N_CB = 10                  # class blocks of 128 (last one padded 98->128)
CPAD = N_CB * 128          # 1280
XSCALE = 0.5               # host: x *= XSCALE, W *= 1/XSCALE (cancels)

# class-block tiles: (first block, n blocks). The dependent tail
# (dma-sem -> matmul -> cast -> out issue -> out DMA -> sem) hangs off the
# LAST wg DMA, so later tiles are smaller; bigger tiles stream in halves
# (by contract pairs) so their matmuls don't all wait for the full DMA.
CB_TILES = [(0, 4), (4, 4), (8, 1), (9, 1)]
WG_SPLITS = [[(0, 4), (4, 8)], [(0, 4), (4, 8)], [(0, 8)], [(0, 6), (6, 8)]]
N_WARM = 6                 # PE warm-up matmuls (ramps pstate before logits)

# aux tensor layout (fp8): [bias (P*K) | bsum (CPAD) | ones (128)]
AUX_BIAS, AUX_BSUM, AUX_ONES = 0, P * K, P * K + CPAD
AUX_LEN = P * K + CPAD + 128

_cached = {}


def _build_program():
    if "nc" in _cached:
        return _cached["nc"]

    nc = bacc.Bacc("TRN2", target_bir_lowering=False, debug=False,
                   num_devices=N_CORES)
    dt = mybir.dt
    DRI = mybir.MatmulPerfMode.DoubleRowSwInterleave

    xt_d = nc.dram_tensor("xt", [128, NT, 128], dt.float8e4, kind="ExternalInput")
    wk_d = nc.dram_tensor("wk", [128, NT, K], dt.float8e4, kind="ExternalInput")
    # wg, interleaved dual-row layout, c-tile-major:
    # per partition j: for each tile: [pair pi][block cb][A/B interleave 256B]
    wg_d = nc.dram_tensor("wg", [128, NPAIR * N_CB * 256], dt.float8e4,
                          kind="ExternalInput")
    aux_d = nc.dram_tensor("aux", [1, AUX_LEN], dt.float8e4, kind="ExternalInput")
    # class-major output: [class-in-block, block, batch]
    out_d = nc.dram_tensor("out", [128, N_CB, 128], dt.bfloat16,
                           kind="ExternalOutput")
    # raw per-(b,p) exp-sums; the host finishes LSE[b] = sum_p ln(sums[b,p])
    sums_d = nc.dram_tensor("sums", [128, P], dt.float32, kind="ExternalOutput")

    with tile.TileContext(nc) as tc:
        with (
            tc.tile_pool(name="const", bufs=1) as cpool,
            tc.tile_pool(name="psum", bufs=8, space="PSUM") as ppool,
        ):
            xt_sb = cpool.tile([128, NT, 128], dt.float8e4)
            wk_sb = cpool.tile([128, NT, K], dt.float8e4)
            aux_sb = cpool.tile([1, AUX_LEN], dt.float8e4)
            wg_ts = [cpool.tile([128, NPAIR, nb, 2, 128], dt.float8e4,
                                name=f"wg{i}")
                     for i, (cb0, nb) in enumerate(CB_TILES)]
            exp_sb = cpool.tile([128, P, K], dt.bfloat16)
            sums_sb = cpool.tile([128, P], dt.float32)
            zscr_sb = cpool.tile([1, 640], dt.float8e4)
            ot0 = cpool.tile([128, 4, 128], dt.bfloat16)
            ot1 = cpool.tile([128, 4, 128], dt.bfloat16)
            ot23 = cpool.tile([128, 2, 128], dt.bfloat16)

            bias = lambda lo, n: aux_sb[:, AUX_BIAS + lo:AUX_BIAS + lo + n]
            bsum = lambda lo, n: aux_sb[:, AUX_BSUM + lo:AUX_BSUM + lo + n]
            ones_ap = aux_sb[:, AUX_ONES:AUX_ONES + 128]

            # preload the activation table set that holds BOTH exp and ln so
            # the auto-inserted per-function loads (1283ns each) are skipped
            nc.scalar.add_instruction(mybir.InstLoadActFuncSet(
                name=nc.get_next_instruction_name(), ins=[], outs=[],
                act_func_set_id=6))

            # --- input DMAs. The big stream rides SP in exact transfer
            # order (the shared DMA unit serves descriptors in ready-order):
            # xt+wk first (logits chain), then the wg c-tiles big-to-small.
            # The tiny aux is issued on Pool/SWDGE concurrently — its
            # descriptors come ready between xt's and wk's, so its 25ns
            # transfer slots in harmlessly without burning an SP issue slot
            # or an HWDGE generation slot. ---
            def wg_dma(ti, p0, p1):
                cb0, nb = CB_TILES[ti]
                off = sum(NPAIR * n * 256 for _, n in CB_TILES[:ti])
                nc.sync.dma_start(
                    wg_ts[ti][:, p0:p1, :, :, :],
                    wg_d[:, off + p0 * nb * 256: off + p1 * nb * 256]
                    .rearrange("p (a b c d) -> p a b c d",
                               a=p1 - p0, b=nb, c=2, d=128))

            nc.sync.dma_start(xt_sb[:], xt_d[:])
            nc.gpsimd.dma_start(aux_sb[:], aux_d[:])
            nc.sync.dma_start(wk_sb[:], wk_d[:])
            for ti in range(4):
                for (p0, p1) in WG_SPLITS[ti]:
                    wg_dma(ti, p0, p1)

            # --- PE warm-up: zero-input matmuls ramp the tensor engine's
            # pstate while the first DMAs are in flight, so the real matmuls
            # run at full clock ---
            nc.vector.memset(zscr_sb[:], 0.0)
            warm_ps = ppool.tile([128, 512], dt.float32, tag="ps")
            for _ in range(N_WARM):
                nc.tensor.matmul(warm_ps[:], zscr_sb[:, 0:128],
                                 zscr_sb[:, 128:640], start=True, stop=True)

            # --- logits -> exp (each psum tile holds 4 p's); x@W matmul
            # first (needs only xt+wk at ~4.2us), bias rank-1 second (aux
            # lands ~3.6us, so no PE stall). exp on ACT; segmented sums on
            # DVE; ln + final sum happen on the HOST (the sums ship as their
            # own tiny output, so nothing downstream waits on them). ---
            for j in range(P // 4):
                ps = ppool.tile([128, 512], dt.float32, tag="ps")
                for q in range(4):
                    p = 4 * j + q
                    t, h = p // 2, p % 2
                    reg = ps[:, q * K:(q + 1) * K]
                    nc.tensor.matmul(reg,
                                     xt_sb[h * 64:h * 64 + 64, t, :],
                                     wk_sb[h * 64:h * 64 + 64, t, :],
                                     start=True, stop=False)
                    nc.tensor.matmul(reg, ones_ap, bias(p * K, K),
                                     start=False, stop=True)
                nc.scalar.activation(exp_sb[:, 4 * j:4 * j + 4, :],
                                     ps[:, 0:4 * K],
                                     mybir.ActivationFunctionType.Exp)
                nc.vector.tensor_reduce(sums_sb[:, 4 * j:4 * j + 4],
                                        exp_sb[:, 4 * j:4 * j + 4, :],
                                        axis=mybir.AxisListType.X,
                                        op=mybir.AluOpType.add)

            # --- main fp8 dual-row matmul over the C-shard, c-tile outer.
            # Per psum bank: rank-1 bsum matmuls seed each 128-class block
            # (start=True only on the bank's first instruction — start zeroes
            # the whole 2KB zero-region), then interleaved-wg DoubleRows.
            # psum->sbuf casts alternate DVE/ACT; out DMAs spread across
            # ACT/Pool/SP sequencers so the tail issues don't serialize. ---
            for ti, (cb0, nb) in enumerate(CB_TILES):
                wt = wg_ts[ti]
                ps = ppool.tile([128, 512], dt.float32, tag="ps")
                for cb in range(nb):
                    nc.tensor.matmul(ps[:, cb * 128:(cb + 1) * 128],
                                     bsum((cb0 + cb) * 128, 128), ones_ap,
                                     start=(cb == 0), stop=False,
                                     skip_group_check=True)
                for pi in range(NPAIR):
                    for cb in range(nb):
                        nc.tensor.matmul(
                            ps[:, cb * 128:(cb + 1) * 128],
                            wt[:, pi, cb, :, :],
                            xt_sb[:, 2 * pi:2 * pi + 2, :],
                            start=False,
                            stop=(pi == NPAIR - 1 and cb == nb - 1),
                            perf_mode=DRI, skip_group_check=True)
                # psum->sbuf casts alternate DVE/ACT; out issues spread
                # across sequencers/DGE paths so the tail descriptor
                # generations don't serialize: sums + out0 ride Pool/SWDGE
                # (ready earliest), out1 rides ACT, and tiles 2+3 ship as
                # ONE SP DMA (adjacent blocks in one buffer, innermost run
                # 512B — no small-elem penalty)
                if ti == 0:
                    nc.vector.tensor_scalar_add(ot0[:], ps[:, 0:512], 0.0)
                    nc.gpsimd.dma_start(sums_d[:], sums_sb[:])
                    nc.gpsimd.dma_start(out_d[:, 0:4, :], ot0[:])
                elif ti == 1:
                    nc.scalar.activation(ot1[:], ps[:, 0:512],
                                         mybir.ActivationFunctionType.Copy)
                    nc.scalar.dma_start(out_d[:, 4:8, :], ot1[:])
                elif ti == 2:
                    nc.scalar.activation(ot23[:, 0, :], ps[:, 0:128],
                                         mybir.ActivationFunctionType.Copy)
                else:
                    nc.vector.tensor_scalar_add(ot23[:, 1, :], ps[:, 0:128],
                                                0.0)
                    nc.sync.dma_start(out_d[:, 8:10, :], ot23[:])

    nc.compile()
    _cached["nc"] = nc
    return nc


def _prep_inputs(x, W, b, idx):
    """Host-side data prep -> per-core input maps."""
    x = np.asarray(x, dtype=np.float32) * XSCALE
    W = np.asarray(W, dtype=np.float32) * (1.0 / XSCALE)
    b = np.asarray(b, dtype=np.float32)
    idx = np.asarray(idx, dtype=np.int64)

    # x^T in (s_local, chunk, b) layout
    xt = np.ascontiguousarray(
        x.T.reshape(NT, 128, B).transpose(1, 0, 2)).astype(F8)

    # packed per-pair weights for the logits path: (128, NT, K)
    wk = np.empty((128, NT, K), dtype=np.float32)
    for t in range(NT):
        wk[0:64, t, :] = W[2 * t].T
        wk[64:128, t, :] = W[2 * t + 1].T
    wk = wk.astype(F8)

    # gathered big weight matrix: Wg[(p,s), c] = W[p, idx[p,c], s]
    Wg = W[np.arange(P)[:, None], idx]            # (P, C, S)
    Wg = np.ascontiguousarray(Wg.transpose(0, 2, 1)).reshape(P * S, C)
    bsum_full = b[np.arange(P)[:, None], idx].sum(axis=0)   # (C,)

    aux_base = np.zeros((1, AUX_LEN), dtype=np.float32)
    aux_base[0, AUX_BIAS:AUX_BIAS + P * K] = b.reshape(-1)
    aux_base[0, AUX_ONES:AUX_ONES + 128] = 1.0

    in_maps = []
    for m in range(N_CORES):
        Wgp = np.zeros((P * S, CPAD), dtype=np.float32)
        Wgp[:, :CS] = Wg[:, m * CS:(m + 1) * CS]
        # dual-row interleave: per (j, pair, block) a 256-byte token
        # [A_127, B_127, A_126, B_126, ..., A_0, B_0] where A/B are the
        # pair's two contract chunks and columns are stored reversed
        M4 = Wgp.reshape(NPAIR, 2, 128, N_CB, 128)   # [pi, q, j, cb, cc]
        rev = M4[:, :, :, :, ::-1]                    # reverse class-in-block
        inter = rev.transpose(2, 0, 3, 4, 1)          # [j, pi, cb, cc_r, q]
        inter = np.ascontiguousarray(inter).reshape(128, NPAIR, N_CB, 256)
        # c-tile-major flat layout, [pair][block] inside each tile
        wg = np.concatenate(
            [np.ascontiguousarray(inter[:, :, cb0:cb0 + nb, :]
                                  ).reshape(128, NPAIR * nb * 256)
             for (cb0, nb) in CB_TILES], axis=1).astype(F8)
        aux = aux_base.copy()
        aux[0, AUX_BSUM:AUX_BSUM + CS] = bsum_full[m * CS:(m + 1) * CS]
        in_maps.append({"xt": xt, "wk": wk, "wg": wg, "aux": aux.astype(F8)})
    return in_maps


def kernel(x, W, b, partitionings):
    nc = _build_program()
    in_maps = _prep_inputs(x, W, b, partitionings)
    res = run_bass_kernel_spmd(nc, in_maps, list(range(N_CORES)))
    sums = np.asarray(res.results[0]["sums"]).astype(np.float32)  # (128, P)
    lse = np.log(sums).sum(axis=1, keepdims=True)                 # (128, 1)
    cores = []
    for m in range(N_CORES):
        o = np.asarray(res.results[m]["out"]).astype(np.float32)  # (128,10,128)
        # [class-in-block, block, batch] -> (batch, class)
        cores.append(o.transpose(2, 1, 0).reshape(128, CPAD)[:, :CS])
    out = np.concatenate(cores, axis=1)
    return out - lse



# revision 9
# speedup vs baseline: 1.0370x; 1.0370x over previous
"""Trainium2 Bass kernel for nn_CombinatorialClassifierSplit.

Reference computation:
    xr = x.reshape(B, P, S)
    logits = einsum('bps,pks', xr, W) + b          # (B, P, K)
    logp = log_softmax(logits, axis=2)
    out[b, c] = sum_p logp[b, p, idx[p, c]]        # (B, C)

Key restructuring: since idx doesn't depend on b,
    out[b, c] = sum_p logits[b, p, idx[p, c]] - LSE[b]
with LSE[b] = sum_p logsumexp_k(logits[b, p, :]).  The first term is a
plain matmul  M = x_flat @ Wg  where Wg[(p,s), c] = W[p, idx[p,c], s] is a
host-side gather of the *static* index tensor, plus a host-side rank-1
bsum[c] = sum_p b[p, idx[p,c]] correction.  Classes are sharded 8 ways
(CS = 1250/core, zero padding: blocks of 98 + 9x128).

Per core the device computes:
  - the LSE partials for ONLY its 4 partitionings (the p-dimension of the
    softmax stats is data-parallel across cores, killing the 8x replicated
    logits work):  x@W -> +bias -> exp (ACT) -> row-sums (DVE) -> `sums`
    output; the host finishes LSE[b] = sum over all cores' ln(sums).
  - the big matmul (contract 2048) in fp8 DoubleRowSwInterleave mode,
    streamed tile-by-tile (wg is the dominant 2.56MB DMA), with the class
    tiles ordered big->small so the dependent tail (last wg chunk -> +900ns
    DMA sem -> last 2 matmuls -> cast -> out DMA) hangs off a single
    128-class block.
  - psum->sbuf bf16 casts alternate DVE/ACT; outputs ride three HWDGE DMAs
    whose descriptor generations are spread across SP/ACT sequencers so the
    shared HWDGE unit never serializes into the critical tail.
  - zero-operand PE filler matmuls pad every DMA-wait gap so the tensor
    engine's p-state stays ramped (27ns vs 53ns per DoubleRow in the tail).

All matmul operands are fp8e4 (e4m3): x is pre-scaled by 1/2 and W by 2
on the host (the scales cancel in x@W), which centers both operand
distributions inside e4m3's normal range.  M ~ N(0, 5.7) so bf16 output
rounding is ~0.03 versus an error budget of ~3.8.  The bias gather bsum
and the -LSE shift are applied on the host in fp32.
"""

import numpy as np
import ml_dtypes

import concourse.bacc as bacc
import concourse.tile as tile
from concourse import mybir
from concourse.bass_utils import run_bass_kernel_spmd

F8 = ml_dtypes.float8_e4m3
BF16 = ml_dtypes.bfloat16

B, P, K, S, C = 128, 32, 100, 64, 10000
N_CORES = 8
CS = C // N_CORES          # 1250 classes per core
NT = (P * S) // 128        # 16 contract chunks of 128
NPAIR = NT // 2            # DoubleRow processes chunk pairs
PL = P // N_CORES          # 4 local partitionings for the LSE path
TL = PL // 2               # 2 local contract chunks for the LSE path
XSCALE = 0.5               # host: x *= XSCALE, W *= 1/XSCALE (cancels)

# class blocks: 10 uniform 128-wide blocks; block 0 holds the core's first
# 98 classes + 30 zero-pad columns (DR Ldweights requires 256 active cols,
# so ragged blocks are illegal).  CPAD = 1280 per core.
BLK_W = [128] * 10
CPAD = 1280

# class tiles: (name, [block indices], [(pair_lo, pair_hi) DMA splits])
# Stream order == list order; the LAST tile is a single 128-class block and
# its final pair ships alone so only 2 DoubleRows + one cast trail the
# +900ns semaphore of the last wg byte.
TILES = [
    ("t0", [0],          [(0, 8)]),
    ("a",  [1, 2, 3, 4], [(0, 4), (4, 8)]),
    ("b",  [5, 6],       [(0, 8)]),
    ("c",  [7],          [(0, 8)]),
    ("d",  [8],          [(0, 8)]),
    ("z",  [9],          [(0, 6), (6, 8)]),
]

# aux tensor layout (fp8): [bias (PL*K) | ones (128)]
AUX_BIAS, AUX_ONES = 0, PL * K
AUX_LEN = PL * K + 128
# xtwk: 18 chunks of 128 cols: [x^T chunks 0-15 | wk local chunks 16-17
# (K=100 padded to 128)]
XTWK_NC = NT + TL
XTWK_LEN = XTWK_NC * 128

WG_LEN = NPAIR * 2 * CPAD               # 20480 fp8 bytes per partition

_cached = {}


def _build_program():
    if "nc" in _cached:
        return _cached["nc"]

    nc = bacc.Bacc("TRN2", target_bir_lowering=False, debug=False,
                   num_devices=N_CORES)
    dt = mybir.dt
    DRI = mybir.MatmulPerfMode.DoubleRowSwInterleave

    xtwk_d = nc.dram_tensor("xtwk", [128, XTWK_NC, 128], dt.float8e4,
                            kind="ExternalInput")
    wg_d = nc.dram_tensor("wg", [128, WG_LEN], dt.float8e4,
                          kind="ExternalInput")
    aux_d = nc.dram_tensor("aux", [1, AUX_LEN], dt.float8e4,
                           kind="ExternalInput")
    # [class-in-block, block, batch] outputs; host transposes back
    outab_d = nc.dram_tensor("outab", [128, 7, 128], dt.bfloat16,
                             kind="ExternalOutput")
    outc_d = nc.dram_tensor("outc", [128, 128], dt.bfloat16,
                            kind="ExternalOutput")
    outdz_d = nc.dram_tensor("outdz", [128, 2, 128], dt.bfloat16,
                             kind="ExternalOutput")
    # raw per-(b,p_local) exp-sums; host: LSE[b] = sum ln(sums[b,:]) over cores
    sums_d = nc.dram_tensor("sums", [128, PL], dt.float32,
                            kind="ExternalOutput")

    with tile.TileContext(nc) as tc:
        with (
            tc.tile_pool(name="const", bufs=1) as cpool,
            tc.tile_pool(name="psum", bufs=8, space="PSUM") as ppool,
        ):
            xtwk_sb = cpool.tile([128, XTWK_NC, 128], dt.float8e4)
            aux_sb = cpool.tile([1, AUX_LEN], dt.float8e4)
            wg_ts = {}
            for name, blks, _sp in TILES:
                nb, w = len(blks), BLK_W[blks[0]]
                wg_ts[name] = cpool.tile([128, NPAIR, nb, 2, w], dt.float8e4,
                                         name=f"wg_{name}")
            exp_sb = cpool.tile([128, PL, K], dt.bfloat16)
            sums_sb = cpool.tile([128, PL], dt.float32)
            zscr_sb = cpool.tile([1, 640], dt.float8e4)
            otab = cpool.tile([128, 7, 128], dt.bfloat16)
            otc = cpool.tile([128, 128], dt.bfloat16)
            otdz = cpool.tile([128, 2, 128], dt.bfloat16)

            bias = lambda lo, n: aux_sb[:, AUX_BIAS + lo:AUX_BIAS + lo + n]
            ones_ap = aux_sb[:, AUX_ONES:AUX_ONES + 128]

            # preload the activation table set holding Exp so the
            # auto-inserted per-function load (1283ns) is skipped
            nc.scalar.add_instruction(mybir.InstLoadActFuncSet(
                name=nc.get_next_instruction_name(), ins=[], outs=[],
                act_func_set_id=6))

            # --- input DMAs, all on SP/HWDGE in exact stream order; the tiny
            # aux rides Pool/SWDGE and slots into a transfer gap ---
            nc.sync.dma_start(xtwk_sb[:], xtwk_d[:])
            nc.gpsimd.dma_start(aux_sb[:], aux_d[:])
            off = 0
            for name, blks, splits in TILES:
                nb, w = len(blks), BLK_W[blks[0]]
                per_pair = nb * 2 * w
                for (p0, p1) in splits:
                    nc.sync.dma_start(
                        wg_ts[name][:, p0:p1, :, :, :],
                        wg_d[:, off + p0 * per_pair: off + p1 * per_pair]
                        .rearrange("p (a b c d) -> p a b c d",
                                   a=p1 - p0, b=nb, c=2, d=w))
                off += NPAIR * per_pair

            # --- PE warm-up: zero-input matmuls ramp the tensor engine's
            # p-state while the first DMAs are in flight ---
            nc.vector.memset(zscr_sb[:], 0.0)
            fill_ps = ppool.tile([128, 128], dt.float32, tag="ps")

            def fillers(n):
                for _ in range(n):
                    nc.tensor.matmul(fill_ps[:], zscr_sb[:, 0:128],
                                     zscr_sb[:, 128:256],
                                     start=True, stop=True,
                                     skip_group_check=True)

            fillers(4)

            # --- LSE partials for the core's own 4 partitionings:
            # logits -> exp (ACT) -> row sums (DVE) -> tiny f32 output.
            # ln + cross-core sum happen on the HOST. ---
            # The host permutes the 16 contract chunks per core so the core's
            # own 2 logits chunks sit at positions 0-1 (the contract sum of
            # the main matmul is order-agnostic; wg rows are permuted to
            # match).  The SPMD program can then use fixed chunk slices.
            psL = ppool.tile([128, PL * K], dt.float32, tag="ps")
            for tt in range(TL):
                for h in range(2):
                    pl = 2 * tt + h
                    reg = psL[:, pl * K:(pl + 1) * K]
                    nc.tensor.matmul(
                        reg,
                        xtwk_sb[h * 64:h * 64 + 64, tt, :],
                        xtwk_sb[h * 64:h * 64 + 64, NT + tt, 0:K],
                        start=True, stop=False)
                    nc.tensor.matmul(reg, ones_ap, bias(pl * K, K),
                                     start=False, stop=True)
            nc.scalar.activation(exp_sb[:], psL[:, 0:PL * K],
                                 mybir.ActivationFunctionType.Exp)
            nc.vector.tensor_reduce(sums_sb[:], exp_sb[:],
                                    axis=mybir.AxisListType.X,
                                    op=mybir.AluOpType.add)
            nc.gpsimd.dma_start(sums_d[:], sums_sb[:])

            fillers(24)

            # --- main fp8 dual-row matmul, tile by tile. Per psum bank the
            # first matmul carries start=True (zeroes the bank); the last DR
            # carries stop. The host applies bsum + (-LSE) afterwards. ---
            ps_t = {}
            for name, blks, _sp in TILES:
                nb, w = len(blks), BLK_W[blks[0]]
                ps_t[name] = ppool.tile([128, nb * 128], dt.float32, tag="ps",
                                        name=f"ps_{name}")

            def tile_drs(name, blks, splits):
                nb, w = len(blks), BLK_W[blks[0]]
                wt, ps = wg_ts[name], ps_t[name]
                for si, (p0, p1) in enumerate(splits):
                    for pi in range(p0, p1):
                        for bi in range(nb):
                            nc.tensor.matmul(
                                ps[0:w, bi * 128:(bi + 1) * 128],
                                wt[:, pi, bi, :, :],
                                xtwk_sb[:, 2 * pi:2 * pi + 2, :],
                                start=(pi == p0 == 0 and bi == 0),
                                stop=(pi == NPAIR - 1 and bi == nb - 1),
                                perf_mode=DRI, skip_group_check=True)

            # t0 tile
            tile_drs("t0", *_tile("t0"))
            fillers(16)
            tile_drs("a", *_tile("a"))
            fillers(16)
            nc.vector.tensor_scalar_add(otab[:, 0, :], ps_t["t0"][:, 0:128],
                                        0.0)
            fillers(8)
            tile_drs("b", *_tile("b"))
            nc.vector.tensor_scalar_add(otab[:, 1:5, :],
                                        ps_t["a"][:, 0:512], 0.0)
            fillers(8)
            tile_drs("c", *_tile("c"))
            nc.scalar.activation(otab[:, 5:7, :], ps_t["b"][:, 0:256],
                                 mybir.ActivationFunctionType.Copy)
            # outab: single HWDGE DMA on ACT for blocks 0-6
            nc.scalar.dma_start(outab_d[:], otab[:])
            fillers(6)
            nc.vector.tensor_scalar_add(otc[:], ps_t["c"][:, 0:128], 0.0)
            nc.sync.dma_start(outc_d[:], otc[:])
            tile_drs("d", *_tile("d"))
            fillers(6)
            nc.scalar.activation(otdz[:, 0, :], ps_t["d"][:, 0:128],
                                 mybir.ActivationFunctionType.Copy)
            tile_drs("z", *_tile("z"))
            nc.vector.tensor_scalar_add(otdz[:, 1, :], ps_t["z"][:, 0:128],
                                        0.0)
            nc.sync.dma_start(outdz_d[:], otdz[:])

    nc.compile()
    _cached["nc"] = nc
    return nc


def _tile(name):
    for n, blks, splits in TILES:
        if n == name:
            return blks, splits
    raise KeyError(name)


def _prep_inputs(x, W, b, idx):
    """Host-side data prep -> per-core input maps."""
    x = np.asarray(x, dtype=np.float32) * XSCALE
    W = np.asarray(W, dtype=np.float32) * (1.0 / XSCALE)
    b = np.asarray(b, dtype=np.float32)
    idx = np.asarray(idx, dtype=np.int64)

    # x^T in (s_local, chunk, b) layout: (128, NT, 128)
    xt = np.ascontiguousarray(
        x.T.reshape(NT, 128, B).transpose(1, 0, 2))

    # gathered big weight matrix: Wg[(p,s), c] = W[p, idx[p,c], s],
    # chunk-major rows: (NT, 128, C)
    Wg = W[np.arange(P)[:, None], idx]            # (P, C, S)
    Wg = np.ascontiguousarray(Wg.transpose(0, 2, 1)).reshape(NT, 128, C)
    bsum_full = b[np.arange(P)[:, None], idx].sum(axis=0)   # (C,)

    aux_base = np.zeros((1, AUX_LEN), dtype=np.float32)
    aux_base[0, AUX_ONES:AUX_ONES + 128] = 1.0

    in_maps = []
    for m in range(N_CORES):
        # per-core chunk permutation: own chunks (2m, 2m+1) first, so the
        # SPMD logits path can address them at fixed positions 0-1
        perm = [2 * m, 2 * m + 1] + [t for t in range(NT)
                                     if t not in (2 * m, 2 * m + 1)]
        xtp = xt[:, perm, :].reshape(128, NT * 128)
        Wgp = Wg[perm].reshape(P * S, C)

        # per-core wk shard: local chunks tt=0,1 are global chunks 2m+tt
        wk = np.zeros((128, TL, 128), dtype=np.float32)
        for tt in range(TL):
            t = 2 * m + tt
            wk[0:64, tt, 0:K] = W[2 * t].T
            wk[64:128, tt, 0:K] = W[2 * t + 1].T
        xtwk = np.concatenate([xtp.reshape(128, NT, 128), wk],
                              axis=1).astype(F8)

        # zero-pad the core's 1250 classes to 1280: block 0 = 98 real + 30
        Wpad = np.zeros((P * S, CPAD), dtype=np.float32)
        Wpad[:, 0:98] = Wgp[:, m * CS:m * CS + 98]
        Wpad[:, 128:] = Wgp[:, m * CS + 98:(m + 1) * CS]

        # per-tile dual-row interleaved wg shard
        segs = []
        for name, blks, _sp in TILES:
            nb, w = len(blks), BLK_W[blks[0]]
            Wblk = Wpad[:, blks[0] * 128:(blks[0] + nb) * 128]
            M4 = Wblk.reshape(NPAIR, 2, 128, nb, w)   # [pi, q, j, bi, cc]
            rev = M4[:, :, :, :, ::-1]
            inter = rev.transpose(2, 0, 3, 4, 1)      # [j, pi, bi, cc_r, q]
            segs.append(np.ascontiguousarray(inter).reshape(128, -1))
        wg = np.concatenate(segs, axis=1).astype(F8)
        assert wg.shape[1] == WG_LEN, wg.shape

        aux = aux_base.copy()
        aux[0, AUX_BIAS:AUX_BIAS + PL * K] = \
            b[PL * m:PL * (m + 1)].reshape(-1)
        in_maps.append({"xtwk": xtwk, "wg": wg, "aux": aux.astype(F8),
                        "_bsum": bsum_full[m * CS:(m + 1) * CS]})
    return in_maps


def kernel(x, W, b, partitionings):
    nc = _build_program()
    in_maps = _prep_inputs(x, W, b, partitionings)
    dev_maps = [{k: v for k, v in im.items() if not k.startswith("_")}
                for im in in_maps]
    res = run_bass_kernel_spmd(nc, dev_maps, list(range(N_CORES)))

    # LSE[b] = sum over all 32 p of ln(exp-sum); each core did 4 p's
    sums = np.concatenate(
        [np.asarray(res.results[m]["sums"]).astype(np.float32)
         for m in range(N_CORES)], axis=1)                    # (128, 32)
    lse = np.log(sums).sum(axis=1, keepdims=True)             # (128, 1)

    cores = []
    for m in range(N_CORES):
        r = res.results[m]
        blkcols = []
        ab = np.asarray(r["outab"]).astype(np.float32)        # (128, 7, 128)
        oc = np.asarray(r["outc"]).astype(np.float32)         # (128, 128)
        dz = np.asarray(r["outdz"]).astype(np.float32)        # (128, 2, 128)
        blkcols.append(ab[0:98, 0, :].T)                      # block 0 (98)
        for k in range(1, 7):
            blkcols.append(ab[:, k, :].T)                     # blocks 1-6
        blkcols.append(oc.T)                                  # block 7
        blkcols.append(dz[:, 0, :].T)                         # block 8
        blkcols.append(dz[:, 1, :].T)                         # block 9
        core_out = np.concatenate(blkcols, axis=1)            # (128, 1250)
        core_out += in_maps[m]["_bsum"][None, :]
        cores.append(core_out)
    out = np.concatenate(cores, axis=1)
    return (out - lse).astype(np.float32)


# revision 16
# speedup vs baseline: 1.0471x; 1.0098x over previous
"""Trainium2 Bass kernel for nn_CombinatorialClassifierSplit.

Reference computation:
    xr = x.reshape(B, P, S)
    logits = einsum('bps,pks', xr, W) + b          # (B, P, K)
    logp = log_softmax(logits, axis=2)
    out[b, c] = sum_p logp[b, p, idx[p, c]]        # (B, C)

Key restructuring: since idx doesn't depend on b,
    out[b, c] = sum_p logits[b, p, idx[p, c]] - LSE[b]
with LSE[b] = sum_p logsumexp_k(logits[b, p, :]).  The first term is a
plain matmul  M = x_flat @ Wg  where Wg[(p,s), c] = W[p, idx[p,c], s] is a
host-side gather of the *static* index tensor, plus a host-side rank-1
bsum[c] = sum_p b[p, idx[p,c]] correction.  Classes are sharded 8 ways
(CS = 1250/core, zero padding: blocks of 98 + 9x128).

Per core the device computes:
  - the LSE partials for ONLY its 4 partitionings (the p-dimension of the
    softmax stats is data-parallel across cores, killing the 8x replicated
    logits work):  x@W -> +bias -> exp (ACT) -> row-sums (DVE) -> `sums`
    output; the host finishes LSE[b] = sum over all cores' ln(sums).
  - the big matmul (contract 2048) in fp8 DoubleRowSwInterleave mode,
    streamed tile-by-tile (wg is the dominant 2.56MB DMA), with the class
    tiles ordered big->small so the dependent tail (last wg chunk -> +900ns
    DMA sem -> last 2 matmuls -> cast -> out DMA) hangs off a single
    128-class block.
  - psum->sbuf bf16 casts alternate DVE/ACT; outputs ride three HWDGE DMAs
    whose descriptor generations are spread across SP/ACT sequencers so the
    shared HWDGE unit never serializes into the critical tail.
  - zero-operand PE filler matmuls pad every DMA-wait gap so the tensor
    engine's p-state stays ramped (27ns vs 53ns per DoubleRow in the tail).

All matmul operands are fp8e4 (e4m3): x is pre-scaled by 1/2 and W by 2
on the host (the scales cancel in x@W), which centers both operand
distributions inside e4m3's normal range.  M ~ N(0, 5.7) so bf16 output
rounding is ~0.03 versus an error budget of ~3.8.  The bias gather bsum
and the -LSE shift are applied on the host in fp32.
"""

import numpy as np
import ml_dtypes

import concourse.bacc as bacc
import concourse.tile as tile
from concourse import mybir
from concourse.bass_utils import run_bass_kernel_spmd

F8 = ml_dtypes.float8_e4m3
BF16 = ml_dtypes.bfloat16

B, P, K, S, C = 128, 32, 100, 64, 10000
N_CORES = 8
CS = C // N_CORES          # 1250 classes per core
NT = (P * S) // 128        # 16 contract chunks of 128
NPAIR = NT // 2            # DoubleRow processes chunk pairs
PL = P // N_CORES          # 4 local partitionings for the LSE path
TL = PL // 2               # 2 local contract chunks for the LSE path
XSCALE = 0.5               # host: x *= XSCALE, W *= 1/XSCALE (cancels)

# class blocks: 10 uniform 128-wide blocks; block 0 holds the core's first
# 98 classes + 30 zero-pad columns (DR Ldweights requires 256 active cols,
# so ragged blocks are illegal).  CPAD = 1280 per core.
BLK_W = [128] * 10
CPAD = 1280

# class tiles: (name, [block indices], [(pair_lo, pair_hi) DMA splits])
# Stream order == list order; the LAST tile is a single 128-class block and
# its final pair ships alone so only 2 DoubleRows + one cast trail the
# +900ns semaphore of the last wg byte.
TILES = [
    ("t0", [0],          [(0, 8)]),
    ("a",  [1, 2, 3, 4], [(0, 2), (2, 4), (4, 6), (6, 8)]),
    ("b",  [5, 6],       [(0, 4), (4, 8)]),
    ("c",  [7],          [(0, 4), (4, 8)]),
    ("d",  [8],          [(0, 8)]),
    ("z",  [9],          [(0, 6), (6, 8)]),
]

# aux tensor layout (fp8): [bias (PL*K) | ones (128)]
AUX_BIAS, AUX_ONES = 0, PL * K
AUX_LEN = PL * K + 128
# xtwk: 18 chunks of 128 cols: [x^T chunks 0-15 | wk local chunks 16-17
# (K=100 padded to 128)]
XTWK_NC = NT + TL
XTWK_LEN = XTWK_NC * 128

WG_LEN = NPAIR * 2 * CPAD               # 20480 fp8 bytes per partition

_cached = {}


def _build_program():
    if "nc" in _cached:
        return _cached["nc"]

    nc = bacc.Bacc("TRN2", target_bir_lowering=False, debug=False,
                   num_devices=N_CORES)
    dt = mybir.dt
    DRI = mybir.MatmulPerfMode.DoubleRowSwInterleave

    xtwk_d = nc.dram_tensor("xtwk", [128, XTWK_NC, 128], dt.float8e4,
                            kind="ExternalInput")
    wg_d = nc.dram_tensor("wg", [128, WG_LEN], dt.float8e4,
                          kind="ExternalInput")
    aux_d = nc.dram_tensor("aux", [1, AUX_LEN], dt.float8e4,
                           kind="ExternalInput")
    # [class-in-block, block, batch] outputs; host transposes back
    outab_d = nc.dram_tensor("outab", [128, 7, 128], dt.bfloat16,
                             kind="ExternalOutput")
    outc_d = nc.dram_tensor("outc", [128, 128], dt.bfloat16,
                            kind="ExternalOutput")
    outdz_d = nc.dram_tensor("outdz", [128, 2, 128], dt.bfloat16,
                             kind="ExternalOutput")
    # raw per-(b,p_local) exp-sums; host: LSE[b] = sum ln(sums[b,:]) over cores
    sums_d = nc.dram_tensor("sums", [128, PL], dt.float32,
                            kind="ExternalOutput")

    with tile.TileContext(nc) as tc:
        with (
            tc.tile_pool(name="const", bufs=1) as cpool,
            tc.tile_pool(name="psum", bufs=8, space="PSUM") as ppool,
        ):
            xtwk_sb = cpool.tile([128, XTWK_NC, 128], dt.float8e4)
            aux_sb = cpool.tile([1, AUX_LEN], dt.float8e4)
            wg_ts = {}
            for name, blks, _sp in TILES:
                nb, w = len(blks), BLK_W[blks[0]]
                wg_ts[name] = cpool.tile([128, NPAIR, nb, 2, w], dt.float8e4,
                                         name=f"wg_{name}")
            exp_sb = cpool.tile([128, PL, K], dt.bfloat16)
            sums_sb = cpool.tile([128, PL], dt.float32)
            zscr_sb = cpool.tile([1, 640], dt.float8e4)
            otab = cpool.tile([128, 7, 128], dt.bfloat16)
            otc = cpool.tile([128, 128], dt.bfloat16)
            otdz = cpool.tile([128, 2, 128], dt.bfloat16)

            bias = lambda lo, n: aux_sb[:, AUX_BIAS + lo:AUX_BIAS + lo + n]
            ones_ap = aux_sb[:, AUX_ONES:AUX_ONES + 128]

            # preload the activation table set holding Exp so the
            # auto-inserted per-function load (1283ns) is skipped
            nc.scalar.add_instruction(mybir.InstLoadActFuncSet(
                name=nc.get_next_instruction_name(), ins=[], outs=[],
                act_func_set_id=6))

            # --- input DMAs, all on SP/HWDGE in exact stream order; the tiny
            # aux rides Pool/SWDGE and slots into a transfer gap ---
            nc.sync.dma_start(xtwk_sb[:], xtwk_d[:])
            nc.gpsimd.dma_start(aux_sb[:], aux_d[:])
            off = 0
            for name, blks, splits in TILES:
                nb, w = len(blks), BLK_W[blks[0]]
                per_pair = nb * 2 * w
                for (p0, p1) in splits:
                    nc.sync.dma_start(
                        wg_ts[name][:, p0:p1, :, :, :],
                        wg_d[:, off + p0 * per_pair: off + p1 * per_pair]
                        .rearrange("p (a b c d) -> p a b c d",
                                   a=p1 - p0, b=nb, c=2, d=w))
                off += NPAIR * per_pair

            # --- PE warm-up: zero-input matmuls ramp the tensor engine's
            # p-state while the first DMAs are in flight ---
            nc.vector.memset(zscr_sb[:], 0.0)
            fill_ps = ppool.tile([128, 128], dt.float32, tag="ps")

            def fillers(n):
                for _ in range(n):
                    nc.tensor.matmul(fill_ps[:], zscr_sb[:, 0:128],
                                     zscr_sb[:, 128:256],
                                     start=True, stop=True,
                                     skip_group_check=True)

            fillers(4)

            # --- LSE partials for the core's own 4 partitionings:
            # logits -> exp (ACT) -> row sums (DVE) -> tiny f32 output.
            # ln + cross-core sum happen on the HOST. ---
            # The host permutes the 16 contract chunks per core so the core's
            # own 2 logits chunks sit at positions 0-1 (the contract sum of
            # the main matmul is order-agnostic; wg rows are permuted to
            # match).  The SPMD program can then use fixed chunk slices.
            psL = ppool.tile([128, PL * K], dt.float32, tag="ps")
            for tt in range(TL):
                for h in range(2):
                    pl = 2 * tt + h
                    reg = psL[:, pl * K:(pl + 1) * K]
                    nc.tensor.matmul(
                        reg,
                        xtwk_sb[h * 64:h * 64 + 64, tt, :],
                        xtwk_sb[h * 64:h * 64 + 64, NT + tt, 0:K],
                        start=True, stop=False)
                    nc.tensor.matmul(reg, ones_ap, bias(pl * K, K),
                                     start=False, stop=True)
            nc.scalar.activation(exp_sb[:], psL[:, 0:PL * K],
                                 mybir.ActivationFunctionType.Exp)
            nc.vector.tensor_reduce(sums_sb[:], exp_sb[:],
                                    axis=mybir.AxisListType.X,
                                    op=mybir.AluOpType.add)
            nc.gpsimd.dma_start(sums_d[:], sums_sb[:])

            fillers(24)

            # --- main fp8 dual-row matmul, tile by tile. Per psum bank the
            # first matmul carries start=True (zeroes the bank); the last DR
            # carries stop. The host applies bsum + (-LSE) afterwards. ---
            ps_t = {}
            for name, blks, _sp in TILES:
                nb, w = len(blks), BLK_W[blks[0]]
                ps_t[name] = ppool.tile([128, nb * 128], dt.float32, tag="ps",
                                        name=f"ps_{name}")

            def tile_drs(name, blks, splits):
                nb, w = len(blks), BLK_W[blks[0]]
                wt, ps = wg_ts[name], ps_t[name]
                for si, (p0, p1) in enumerate(splits):
                    for pi in range(p0, p1):
                        for bi in range(nb):
                            nc.tensor.matmul(
                                ps[0:w, bi * 128:(bi + 1) * 128],
                                wt[:, pi, bi, :, :],
                                xtwk_sb[:, 2 * pi:2 * pi + 2, :],
                                start=(pi == p0 == 0 and bi == 0),
                                stop=(pi == NPAIR - 1 and bi == nb - 1),
                                perf_mode=DRI, skip_group_check=True)

            # t0 tile
            tile_drs("t0", *_tile("t0"))
            fillers(16)
            tile_drs("a", *_tile("a"))
            fillers(16)
            nc.vector.tensor_scalar_add(otab[:, 0, :], ps_t["t0"][:, 0:128],
                                        0.0)
            fillers(8)
            tile_drs("b", *_tile("b"))
            nc.vector.tensor_scalar_add(otab[:, 1:5, :],
                                        ps_t["a"][:, 0:512], 0.0)
            fillers(8)
            tile_drs("c", *_tile("c"))
            nc.scalar.activation(otab[:, 5:7, :], ps_t["b"][:, 0:256],
                                 mybir.ActivationFunctionType.Copy)
            # outab: single HWDGE DMA on ACT for blocks 0-6
            nc.scalar.dma_start(outab_d[:], otab[:])
            fillers(6)
            nc.vector.tensor_scalar_add(otc[:], ps_t["c"][:, 0:128], 0.0)
            nc.sync.dma_start(outc_d[:], otc[:])
            tile_drs("d", *_tile("d"))
            fillers(6)
            nc.scalar.activation(otdz[:, 0, :], ps_t["d"][:, 0:128],
                                 mybir.ActivationFunctionType.Copy)
            tile_drs("z", *_tile("z"))
            nc.vector.tensor_scalar_add(otdz[:, 1, :], ps_t["z"][:, 0:128],
                                        0.0)
            nc.sync.dma_start(outdz_d[:], otdz[:])

    nc.compile()
    _cached["nc"] = nc
    return nc


def _tile(name):
    for n, blks, splits in TILES:
        if n == name:
            return blks, splits
    raise KeyError(name)


def _prep_inputs(x, W, b, idx):
    """Host-side data prep -> per-core input maps."""
    x = np.asarray(x, dtype=np.float32) * XSCALE
    W = np.asarray(W, dtype=np.float32) * (1.0 / XSCALE)
    b = np.asarray(b, dtype=np.float32)
    idx = np.asarray(idx, dtype=np.int64)

    # x^T in (s_local, chunk, b) layout: (128, NT, 128)
    xt = np.ascontiguousarray(
        x.T.reshape(NT, 128, B).transpose(1, 0, 2))

    # gathered big weight matrix: Wg[(p,s), c] = W[p, idx[p,c], s],
    # chunk-major rows: (NT, 128, C)
    Wg = W[np.arange(P)[:, None], idx]            # (P, C, S)
    Wg = np.ascontiguousarray(Wg.transpose(0, 2, 1)).reshape(NT, 128, C)
    bsum_full = b[np.arange(P)[:, None], idx].sum(axis=0)   # (C,)

    aux_base = np.zeros((1, AUX_LEN), dtype=np.float32)
    aux_base[0, AUX_ONES:AUX_ONES + 128] = 1.0

    in_maps = []
    for m in range(N_CORES):
        # per-core chunk permutation: own chunks (2m, 2m+1) first, so the
        # SPMD logits path can address them at fixed positions 0-1
        perm = [2 * m, 2 * m + 1] + [t for t in range(NT)
                                     if t not in (2 * m, 2 * m + 1)]
        xtp = xt[:, perm, :].reshape(128, NT * 128)
        Wgp = Wg[perm].reshape(P * S, C)

        # per-core wk shard: local chunks tt=0,1 are global chunks 2m+tt
        wk = np.zeros((128, TL, 128), dtype=np.float32)
        for tt in range(TL):
            t = 2 * m + tt
            wk[0:64, tt, 0:K] = W[2 * t].T
            wk[64:128, tt, 0:K] = W[2 * t + 1].T
        xtwk = np.concatenate([xtp.reshape(128, NT, 128), wk],
                              axis=1).astype(F8)

        # zero-pad the core's 1250 classes to 1280: block 0 = 98 real + 30
        Wpad = np.zeros((P * S, CPAD), dtype=np.float32)
        Wpad[:, 0:98] = Wgp[:, m * CS:m * CS + 98]
        Wpad[:, 128:] = Wgp[:, m * CS + 98:(m + 1) * CS]

        # per-tile dual-row interleaved wg shard
        segs = []
        for name, blks, _sp in TILES:
            nb, w = len(blks), BLK_W[blks[0]]
            Wblk = Wpad[:, blks[0] * 128:(blks[0] + nb) * 128]
            M4 = Wblk.reshape(NPAIR, 2, 128, nb, w)   # [pi, q, j, bi, cc]
            rev = M4[:, :, :, :, ::-1]
            inter = rev.transpose(2, 0, 3, 4, 1)      # [j, pi, bi, cc_r, q]
            segs.append(np.ascontiguousarray(inter).reshape(128, -1))
        wg = np.concatenate(segs, axis=1).astype(F8)
        assert wg.shape[1] == WG_LEN, wg.shape

        aux = aux_base.copy()
        aux[0, AUX_BIAS:AUX_BIAS + PL * K] = \
            b[PL * m:PL * (m + 1)].reshape(-1)
        in_maps.append({"xtwk": xtwk, "wg": wg, "aux": aux.astype(F8),
                        "_bsum": bsum_full[m * CS:(m + 1) * CS]})
    return in_maps


def kernel(x, W, b, partitionings):
    nc = _build_program()
    in_maps = _prep_inputs(x, W, b, partitionings)
    dev_maps = [{k: v for k, v in im.items() if not k.startswith("_")}
                for im in in_maps]
    res = run_bass_kernel_spmd(nc, dev_maps, list(range(N_CORES)))

    # LSE[b] = sum over all 32 p of ln(exp-sum); each core did 4 p's
    sums = np.concatenate(
        [np.asarray(res.results[m]["sums"]).astype(np.float32)
         for m in range(N_CORES)], axis=1)                    # (128, 32)
    lse = np.log(sums).sum(axis=1, keepdims=True)             # (128, 1)

    cores = []
    for m in range(N_CORES):
        r = res.results[m]
        blkcols = []
        ab = np.asarray(r["outab"]).astype(np.float32)        # (128, 7, 128)
        oc = np.asarray(r["outc"]).astype(np.float32)         # (128, 128)
        dz = np.asarray(r["outdz"]).astype(np.float32)        # (128, 2, 128)
        blkcols.append(ab[0:98, 0, :].T)                      # block 0 (98)
        for k in range(1, 7):
            blkcols.append(ab[:, k, :].T)                     # blocks 1-6
        blkcols.append(oc.T)                                  # block 7
        blkcols.append(dz[:, 0, :].T)                         # block 8
        blkcols.append(dz[:, 1, :].T)                         # block 9
        core_out = np.concatenate(blkcols, axis=1)            # (128, 1250)
        core_out += in_maps[m]["_bsum"][None, :]
        cores.append(core_out)
    out = np.concatenate(cores, axis=1)
    return (out - lse).astype(np.float32)


# revision 17
# speedup vs baseline: 1.0594x; 1.0117x over previous
"""Trainium2 Bass kernel for nn_CombinatorialClassifierSplit.

Reference computation:
    xr = x.reshape(B, P, S)
    logits = einsum('bps,pks', xr, W) + b          # (B, P, K)
    logp = log_softmax(logits, axis=2)
    out[b, c] = sum_p logp[b, p, idx[p, c]]        # (B, C)

Key restructuring: since idx doesn't depend on b,
    out[b, c] = sum_p logits[b, p, idx[p, c]] - LSE[b]
with LSE[b] = sum_p logsumexp_k(logits[b, p, :]).  The first term is a
plain matmul  M = x_flat @ Wg  where Wg[(p,s), c] = W[p, idx[p,c], s] is a
host-side gather of the *static* index tensor, plus a host-side rank-1
bsum[c] = sum_p b[p, idx[p,c]] correction.  Classes are sharded 8 ways
(CS = 1250/core, zero padding: blocks of 98 + 9x128).

Per core the device computes:
  - the LSE partials for ONLY its 4 partitionings (the p-dimension of the
    softmax stats is data-parallel across cores, killing the 8x replicated
    logits work):  x@W -> +bias -> exp (ACT) -> row-sums (DVE) -> `sums`
    output; the host finishes LSE[b] = sum over all cores' ln(sums).
  - the big matmul (contract 2048) in fp8 DoubleRowSwInterleave mode,
    streamed tile-by-tile (wg is the dominant 2.56MB DMA), with the class
    tiles ordered big->small so the dependent tail (last wg chunk -> +900ns
    DMA sem -> last 2 matmuls -> cast -> out DMA) hangs off a single
    128-class block.
  - psum->sbuf bf16 casts alternate DVE/ACT; outputs ride three HWDGE DMAs
    whose descriptor generations are spread across SP/ACT sequencers so the
    shared HWDGE unit never serializes into the critical tail.
  - zero-operand PE filler matmuls pad every DMA-wait gap so the tensor
    engine's p-state stays ramped (27ns vs 53ns per DoubleRow in the tail).

All matmul operands are fp8e4 (e4m3): x is pre-scaled by 1/2 and W by 2
on the host (the scales cancel in x@W), which centers both operand
distributions inside e4m3's normal range.  M ~ N(0, 5.7) so bf16 output
rounding is ~0.03 versus an error budget of ~3.8.  The bias gather bsum
and the -LSE shift are applied on the host in fp32.
"""

import numpy as np
import ml_dtypes

import concourse.bacc as bacc
import concourse.tile as tile
from concourse import mybir
from concourse.bass_utils import run_bass_kernel_spmd

F8 = ml_dtypes.float8_e4m3
BF16 = ml_dtypes.bfloat16

B, P, K, S, C = 128, 32, 100, 64, 10000
N_CORES = 8
CS = C // N_CORES          # 1250 classes per core
NT = (P * S) // 128        # 16 contract chunks of 128
NPAIR = NT // 2            # DoubleRow processes chunk pairs
PL = P // N_CORES          # 4 local partitionings for the LSE path
TL = PL // 2               # 2 local contract chunks for the LSE path
XSCALE = 0.5               # host: x *= XSCALE, W *= 1/XSCALE (cancels)

# class blocks: 10 uniform 128-wide blocks; block 0 holds the core's first
# 98 classes + 30 zero-pad columns (DR Ldweights requires 256 active cols,
# so ragged blocks are illegal).  CPAD = 1280 per core.
BLK_W = [128] * 10
CPAD = 1280

# class tiles: (name, [block indices], [(pair_lo, pair_hi) DMA splits])
# Stream order == list order; the LAST tile is a single 128-class block and
# its final pair ships alone so only 2 DoubleRows + one cast trail the
# +900ns semaphore of the last wg byte.
TILES = [
    ("t0", [0],          [(0, 8)]),
    ("a",  [1, 2, 3, 4], [(0, 2), (2, 4), (4, 6), (6, 8)]),
    ("b",  [5, 6],       [(0, 4), (4, 8)]),
    ("c",  [7],          [(0, 4), (4, 8)]),
    ("d",  [8],          [(0, 8)]),
    ("z",  [9],          [(0, 6), (6, 8)]),
]

# aux tensor layout (fp8): [bias (PL*K) | ones (128)]
AUX_BIAS, AUX_ONES = 0, PL * K
AUX_LEN = PL * K + 128
# xtwk: 18 chunks of 128 cols: [x^T chunks 0-15 | wk local chunks 16-17
# (K=100 padded to 128)]
XTWK_NC = NT + TL
XTWK_LEN = XTWK_NC * 128

WG_LEN = NT * 98 + NPAIR * 2 * 9 * 128  # 20000 fp8 bytes per partition

_cached = {}


def _build_program():
    if "nc" in _cached:
        return _cached["nc"]

    nc = bacc.Bacc("TRN2", target_bir_lowering=False, debug=False,
                   num_devices=N_CORES)
    dt = mybir.dt
    DRI = mybir.MatmulPerfMode.DoubleRowSwInterleave

    xtwk_d = nc.dram_tensor("xtwk", [128, XTWK_NC, 128], dt.float8e4,
                            kind="ExternalInput")
    wg_d = nc.dram_tensor("wg", [128, WG_LEN], dt.float8e4,
                          kind="ExternalInput")
    aux_d = nc.dram_tensor("aux", [1, AUX_LEN], dt.float8e4,
                           kind="ExternalInput")
    # [class-in-block, block, batch] outputs; host transposes back
    outab_d = nc.dram_tensor("outab", [128, 7, 128], dt.bfloat16,
                             kind="ExternalOutput")
    outc_d = nc.dram_tensor("outc", [128, 128], dt.bfloat16,
                            kind="ExternalOutput")
    outdz_d = nc.dram_tensor("outdz", [128, 2, 128], dt.bfloat16,
                             kind="ExternalOutput")
    # raw per-(b,p_local) exp-sums; host: LSE[b] = sum ln(sums[b,:]) over cores
    sums_d = nc.dram_tensor("sums", [128, PL], dt.float32,
                            kind="ExternalOutput")

    with tile.TileContext(nc) as tc:
        with (
            tc.tile_pool(name="const", bufs=1) as cpool,
            tc.tile_pool(name="psum", bufs=8, space="PSUM") as ppool,
        ):
            xtwk_sb = cpool.tile([128, XTWK_NC, 128], dt.float8e4)
            aux_sb = cpool.tile([1, AUX_LEN], dt.float8e4)
            wg_ts = {}
            for name, blks, _sp in TILES:
                if name == "t0":
                    # plain (non-DR) layout: [j, chunk, class], 98 real classes
                    wg_ts[name] = cpool.tile([128, NT, 98], dt.float8e4,
                                             name=f"wg_{name}")
                    continue
                nb, w = len(blks), BLK_W[blks[0]]
                wg_ts[name] = cpool.tile([128, NPAIR, nb, 2, w], dt.float8e4,
                                         name=f"wg_{name}")
            exp_sb = cpool.tile([128, PL, K], dt.bfloat16)
            sums_sb = cpool.tile([128, PL], dt.float32)
            zscr_sb = cpool.tile([1, 640], dt.float8e4)
            otab = cpool.tile([128, 7, 128], dt.bfloat16)
            otc = cpool.tile([128, 128], dt.bfloat16)
            otdz = cpool.tile([128, 2, 128], dt.bfloat16)

            bias = lambda lo, n: aux_sb[:, AUX_BIAS + lo:AUX_BIAS + lo + n]
            ones_ap = aux_sb[:, AUX_ONES:AUX_ONES + 128]

            # preload the activation table set holding Exp so the
            # auto-inserted per-function load (1283ns) is skipped
            nc.scalar.add_instruction(mybir.InstLoadActFuncSet(
                name=nc.get_next_instruction_name(), ins=[], outs=[],
                act_func_set_id=6))

            # --- input DMAs, all on SP/HWDGE in exact stream order; the tiny
            # aux rides Pool/SWDGE and slots into a transfer gap ---
            nc.sync.dma_start(xtwk_sb[:], xtwk_d[:])
            nc.gpsimd.dma_start(aux_sb[:], aux_d[:])
            off = 0
            for name, blks, splits in TILES:
                if name == "t0":
                    nc.sync.dma_start(
                        wg_ts[name][:],
                        wg_d[:, off:off + NT * 98]
                        .rearrange("p (t c) -> p t c", t=NT, c=98))
                    off += NT * 98
                    continue
                nb, w = len(blks), BLK_W[blks[0]]
                per_pair = nb * 2 * w
                for (p0, p1) in splits:
                    nc.sync.dma_start(
                        wg_ts[name][:, p0:p1, :, :, :],
                        wg_d[:, off + p0 * per_pair: off + p1 * per_pair]
                        .rearrange("p (a b c d) -> p a b c d",
                                   a=p1 - p0, b=nb, c=2, d=w))
                off += NPAIR * per_pair

            # --- PE warm-up: zero-input matmuls ramp the tensor engine's
            # p-state while the first DMAs are in flight ---
            nc.vector.memset(zscr_sb[:], 0.0)
            fill_ps = ppool.tile([128, 128], dt.float32, tag="ps")

            def fillers(n):
                for _ in range(n):
                    nc.tensor.matmul(fill_ps[:], zscr_sb[:, 0:128],
                                     zscr_sb[:, 128:256],
                                     start=True, stop=True,
                                     skip_group_check=True)

            fillers(4)

            # --- LSE partials for the core's own 4 partitionings:
            # logits -> exp (ACT) -> row sums (DVE) -> tiny f32 output.
            # ln + cross-core sum happen on the HOST. ---
            # The host permutes the 16 contract chunks per core so the core's
            # own 2 logits chunks sit at positions 0-1 (the contract sum of
            # the main matmul is order-agnostic; wg rows are permuted to
            # match).  The SPMD program can then use fixed chunk slices.
            psL = ppool.tile([128, PL * K], dt.float32, tag="ps")
            for tt in range(TL):
                for h in range(2):
                    pl = 2 * tt + h
                    reg = psL[:, pl * K:(pl + 1) * K]
                    nc.tensor.matmul(
                        reg,
                        xtwk_sb[h * 64:h * 64 + 64, tt, :],
                        xtwk_sb[h * 64:h * 64 + 64, NT + tt, 0:K],
                        start=True, stop=False)
                    nc.tensor.matmul(reg, ones_ap, bias(pl * K, K),
                                     start=False, stop=True)
            nc.scalar.activation(exp_sb[:], psL[:, 0:PL * K],
                                 mybir.ActivationFunctionType.Exp)
            nc.vector.tensor_reduce(sums_sb[:], exp_sb[:],
                                    axis=mybir.AxisListType.X,
                                    op=mybir.AluOpType.add)
            nc.gpsimd.dma_start(sums_d[:], sums_sb[:])

            fillers(24)

            # --- main fp8 dual-row matmul, tile by tile. Per psum bank the
            # first matmul carries start=True (zeroes the bank); the last DR
            # carries stop. The host applies bsum + (-LSE) afterwards. ---
            ps_t = {}
            for name, blks, _sp in TILES:
                nb, w = len(blks), BLK_W[blks[0]]
                ps_t[name] = ppool.tile([128, nb * 128], dt.float32, tag="ps",
                                        name=f"ps_{name}")

            def tile_drs(name, blks, splits):
                wt, ps = wg_ts[name], ps_t[name]
                if name == "t0":
                    # plain fp8 matmuls, flipped: stationary = x chunk
                    # (128 cols, Ldweights-legal), moving = 98-wide block;
                    # psum lands [batch, class]
                    for t in range(NT):
                        nc.tensor.matmul(
                            ps[:, 0:98],
                            xtwk_sb[:, t, :],
                            wt[:, t, :],
                            start=(t == 0), stop=(t == NT - 1),
                            skip_group_check=True)
                    return
                nb, w = len(blks), BLK_W[blks[0]]
                for si, (p0, p1) in enumerate(splits):
                    for pi in range(p0, p1):
                        for bi in range(nb):
                            nc.tensor.matmul(
                                ps[0:w, bi * 128:(bi + 1) * 128],
                                wt[:, pi, bi, :, :],
                                xtwk_sb[:, 2 * pi:2 * pi + 2, :],
                                start=(pi == p0 == 0 and bi == 0),
                                stop=(pi == NPAIR - 1 and bi == nb - 1),
                                perf_mode=DRI, skip_group_check=True)

            # t0 tile
            tile_drs("t0", *_tile("t0"))
            fillers(16)
            tile_drs("a", *_tile("a"))
            fillers(16)
            nc.vector.tensor_scalar_add(otab[:, 0, 0:98], ps_t["t0"][:, 0:98],
                                        0.0)
            fillers(8)
            tile_drs("b", *_tile("b"))
            nc.vector.tensor_scalar_add(otab[:, 1:5, :],
                                        ps_t["a"][:, 0:512], 0.0)
            fillers(8)
            tile_drs("c", *_tile("c"))
            nc.scalar.activation(otab[:, 5:7, :], ps_t["b"][:, 0:256],
                                 mybir.ActivationFunctionType.Copy)
            # outab: single HWDGE DMA on ACT for blocks 0-6
            nc.scalar.dma_start(outab_d[:], otab[:])
            fillers(6)
            nc.vector.tensor_scalar_add(otc[:], ps_t["c"][:, 0:128], 0.0)
            nc.sync.dma_start(outc_d[:], otc[:])
            tile_drs("d", *_tile("d"))
            fillers(6)
            nc.scalar.activation(otdz[:, 0, :], ps_t["d"][:, 0:128],
                                 mybir.ActivationFunctionType.Copy)
            tile_drs("z", *_tile("z"))
            nc.vector.tensor_scalar_add(otdz[:, 1, :], ps_t["z"][:, 0:128],
                                        0.0)
            nc.sync.dma_start(outdz_d[:], otdz[:])

    nc.compile()
    _cached["nc"] = nc
    return nc


def _tile(name):
    for n, blks, splits in TILES:
        if n == name:
            return blks, splits
    raise KeyError(name)


def _prep_inputs(x, W, b, idx):
    """Host-side data prep -> per-core input maps."""
    x = np.asarray(x, dtype=np.float32) * XSCALE
    W = np.asarray(W, dtype=np.float32) * (1.0 / XSCALE)
    b = np.asarray(b, dtype=np.float32)
    idx = np.asarray(idx, dtype=np.int64)

    # x^T in (s_local, chunk, b) layout: (128, NT, 128)
    xt = np.ascontiguousarray(
        x.T.reshape(NT, 128, B).transpose(1, 0, 2))

    # gathered big weight matrix: Wg[(p,s), c] = W[p, idx[p,c], s],
    # chunk-major rows: (NT, 128, C)
    Wg = W[np.arange(P)[:, None], idx]            # (P, C, S)
    Wg = np.ascontiguousarray(Wg.transpose(0, 2, 1)).reshape(NT, 128, C)
    bsum_full = b[np.arange(P)[:, None], idx].sum(axis=0)   # (C,)

    aux_base = np.zeros((1, AUX_LEN), dtype=np.float32)
    aux_base[0, AUX_ONES:AUX_ONES + 128] = 1.0

    in_maps = []
    for m in range(N_CORES):
        # per-core chunk permutation: own chunks (2m, 2m+1) first, so the
        # SPMD logits path can address them at fixed positions 0-1
        perm = [2 * m, 2 * m + 1] + [t for t in range(NT)
                                     if t not in (2 * m, 2 * m + 1)]
        xtp = xt[:, perm, :].reshape(128, NT * 128)
        Wgp = Wg[perm].reshape(P * S, C)

        # per-core wk shard: local chunks tt=0,1 are global chunks 2m+tt
        wk = np.zeros((128, TL, 128), dtype=np.float32)
        for tt in range(TL):
            t = 2 * m + tt
            wk[0:64, tt, 0:K] = W[2 * t].T
            wk[64:128, tt, 0:K] = W[2 * t + 1].T
        xtwk = np.concatenate([xtp.reshape(128, NT, 128), wk],
                              axis=1).astype(F8)

        # per-tile wg shard: t0 = plain chunk-major (98 real classes, no
        # pad); the rest = dual-row interleaved 128-wide blocks
        Wcore = Wgp[:, m * CS:(m + 1) * CS]           # (2048, 1250)
        segs = []
        for name, blks, _sp in TILES:
            if name == "t0":
                t0 = Wcore[:, 0:98].reshape(NT, 128, 98).transpose(1, 0, 2)
                segs.append(np.ascontiguousarray(t0).reshape(128, -1))
                continue
            nb, w = len(blks), BLK_W[blks[0]]
            c_lo = 98 + (blks[0] - 1) * 128
            Wblk = Wcore[:, c_lo:c_lo + nb * w]
            M4 = Wblk.reshape(NPAIR, 2, 128, nb, w)   # [pi, q, j, bi, cc]
            rev = M4[:, :, :, :, ::-1]
            inter = rev.transpose(2, 0, 3, 4, 1)      # [j, pi, bi, cc_r, q]
            segs.append(np.ascontiguousarray(inter).reshape(128, -1))
        wg = np.concatenate(segs, axis=1).astype(F8)
        assert wg.shape[1] == WG_LEN, wg.shape

        aux = aux_base.copy()
        aux[0, AUX_BIAS:AUX_BIAS + PL * K] = \
            b[PL * m:PL * (m + 1)].reshape(-1)
        in_maps.append({"xtwk": xtwk, "wg": wg, "aux": aux.astype(F8),
                        "_bsum": bsum_full[m * CS:(m + 1) * CS]})
    return in_maps


def kernel(x, W, b, partitionings):
    nc = _build_program()
    in_maps = _prep_inputs(x, W, b, partitionings)
    dev_maps = [{k: v for k, v in im.items() if not k.startswith("_")}
                for im in in_maps]
    res = run_bass_kernel_spmd(nc, dev_maps, list(range(N_CORES)))

    # LSE[b] = sum over all 32 p of ln(exp-sum); each core did 4 p's
    sums = np.concatenate(
        [np.asarray(res.results[m]["sums"]).astype(np.float32)
         for m in range(N_CORES)], axis=1)                    # (128, 32)
    lse = np.log(sums).sum(axis=1, keepdims=True)             # (128, 1)

    cores = []
    for m in range(N_CORES):
        r = res.results[m]
        blkcols = []
        ab = np.asarray(r["outab"]).astype(np.float32)        # (128, 7, 128)
        oc = np.asarray(r["outc"]).astype(np.float32)         # (128, 128)
        dz = np.asarray(r["outdz"]).astype(np.float32)        # (128, 2, 128)
        blkcols.append(ab[:, 0, 0:98])                        # block 0 (98)
        for k in range(1, 7):
            blkcols.append(ab[:, k, :].T)                     # blocks 1-6
        blkcols.append(oc.T)                                  # block 7
        blkcols.append(dz[:, 0, :].T)                         # block 8
        blkcols.append(dz[:, 1, :].T)                         # block 9
        core_out = np.concatenate(blkcols, axis=1)            # (128, 1250)
        core_out += in_maps[m]["_bsum"][None, :]
        cores.append(core_out)
    out = np.concatenate(cores, axis=1)
    return (out - lse).astype(np.float32)


# revision 20
# speedup vs baseline: 1.0635x; 1.0039x over previous
"""Trainium2 Bass kernel for nn_CombinatorialClassifierSplit.

Reference computation:
    xr = x.reshape(B, P, S)
    logits = einsum('bps,pks', xr, W) + b          # (B, P, K)
    logp = log_softmax(logits, axis=2)
    out[b, c] = sum_p logp[b, p, idx[p, c]]        # (B, C)

Key restructuring: since idx doesn't depend on b,
    out[b, c] = sum_p logits[b, p, idx[p, c]] - LSE[b]
with LSE[b] = sum_p logsumexp_k(logits[b, p, :]).  The first term is a
plain matmul  M = x_flat @ Wg  where Wg[(p,s), c] = W[p, idx[p,c], s] is a
host-side gather of the *static* index tensor, plus a host-side rank-1
bsum[c] = sum_p b[p, idx[p,c]] correction.  Classes are sharded 8 ways
(CS = 1250/core, zero padding: blocks of 98 + 9x128).

Per core the device computes:
  - the LSE partials for ONLY its 4 partitionings (the p-dimension of the
    softmax stats is data-parallel across cores, killing the 8x replicated
    logits work):  x@W -> +bias -> exp (ACT) -> row-sums (DVE) -> `sums`
    output; the host finishes LSE[b] = sum over all cores' ln(sums).
  - the big matmul (contract 2048) in fp8 DoubleRowSwInterleave mode,
    streamed tile-by-tile (wg is the dominant 2.56MB DMA), with the class
    tiles ordered big->small so the dependent tail (last wg chunk -> +900ns
    DMA sem -> last 2 matmuls -> cast -> out DMA) hangs off a single
    128-class block.
  - psum->sbuf bf16 casts alternate DVE/ACT; outputs ride three HWDGE DMAs
    whose descriptor generations are spread across SP/ACT sequencers so the
    shared HWDGE unit never serializes into the critical tail.
  - zero-operand PE filler matmuls pad every DMA-wait gap so the tensor
    engine's p-state stays ramped (27ns vs 53ns per DoubleRow in the tail).

All matmul operands are fp8e4 (e4m3): x is pre-scaled by 1/2 and W by 2
on the host (the scales cancel in x@W), which centers both operand
distributions inside e4m3's normal range.  M ~ N(0, 5.7) so bf16 output
rounding is ~0.03 versus an error budget of ~3.8.  The bias gather bsum
and the -LSE shift are applied on the host in fp32.
"""

import numpy as np
import ml_dtypes

import concourse.bacc as bacc
import concourse.tile as tile
from concourse import mybir
from concourse.bass_utils import run_bass_kernel_spmd

F8 = ml_dtypes.float8_e4m3
BF16 = ml_dtypes.bfloat16

B, P, K, S, C = 128, 32, 100, 64, 10000
N_CORES = 8
CS = C // N_CORES          # 1250 classes per core
NT = (P * S) // 128        # 16 contract chunks of 128
NPAIR = NT // 2            # DoubleRow processes chunk pairs
PL = P // N_CORES          # 4 local partitionings for the LSE path
TL = PL // 2               # 2 local contract chunks for the LSE path
XSCALE = 0.5               # host: x *= XSCALE, W *= 1/XSCALE (cancels)

# class blocks: 10 uniform 128-wide blocks; block 0 holds the core's first
# 98 classes + 30 zero-pad columns (DR Ldweights requires 256 active cols,
# so ragged blocks are illegal).  CPAD = 1280 per core.
BLK_W = [128] * 10
CPAD = 1280

# class tiles: (name, [block indices], [(pair_lo, pair_hi) DMA splits])
# Stream order == list order; the LAST tile is a single 128-class block and
# its final pair ships alone so only 2 DoubleRows + one cast trail the
# +900ns semaphore of the last wg byte.
TILES = [
    ("t0", [0],          [(0, 8)]),
    ("a",  [1, 2, 3, 4], [(0, 2), (2, 4), (4, 6), (6, 8)]),
    ("b",  [5, 6],       [(0, 4), (4, 8)]),
    ("c",  [7],          [(0, 8)]),
    ("d",  [8],          [(0, 8)]),
    ("z",  [9],          [(0, 6), (6, 8)]),
]

# aux tensor layout (fp8): [bias (PL*K) | ones (128)]
AUX_BIAS, AUX_ONES = 0, PL * K
AUX_LEN = PL * K + 128
# xtwk: 18 chunks of 128 cols: [x^T chunks 0-15 | wk local chunks 16-17
# (K=100 padded to 128)]
XTWK_NC = NT + TL
XTWK_LEN = XTWK_NC * 128

WG_LEN = NT * 98 + NPAIR * 2 * 9 * 128  # 20000 fp8 bytes per partition

_cached = {}


def _build_program():
    if "nc" in _cached:
        return _cached["nc"]

    nc = bacc.Bacc("TRN2", target_bir_lowering=False, debug=False,
                   num_devices=N_CORES)
    dt = mybir.dt
    DRI = mybir.MatmulPerfMode.DoubleRowSwInterleave

    xtwk_d = nc.dram_tensor("xtwk", [128, XTWK_NC, 128], dt.float8e4,
                            kind="ExternalInput")
    wg_d = nc.dram_tensor("wg", [128, WG_LEN], dt.float8e4,
                          kind="ExternalInput")
    aux_d = nc.dram_tensor("aux", [1, AUX_LEN], dt.float8e4,
                           kind="ExternalInput")
    # [class-in-block, block, batch] outputs; host transposes back
    # 8th block slot carries the (b,p_local) exp-sums in bf16
    outab_d = nc.dram_tensor("outab", [128, 8, 128], dt.bfloat16,
                             kind="ExternalOutput")
    outc_d = nc.dram_tensor("outc", [128, 128], dt.bfloat16,
                            kind="ExternalOutput")
    outdz_d = nc.dram_tensor("outdz", [128, 2, 128], dt.bfloat16,
                             kind="ExternalOutput")

    with tile.TileContext(nc) as tc:
        with (
            tc.tile_pool(name="const", bufs=1) as cpool,
            tc.tile_pool(name="psum", bufs=8, space="PSUM") as ppool,
        ):
            xtwk_sb = cpool.tile([128, XTWK_NC, 128], dt.float8e4)
            aux_sb = cpool.tile([1, AUX_LEN], dt.float8e4)
            wg_ts = {}
            for name, blks, _sp in TILES:
                if name == "t0":
                    # plain (non-DR) layout: [j, chunk, class], 98 real classes
                    wg_ts[name] = cpool.tile([128, NT, 98], dt.float8e4,
                                             name=f"wg_{name}")
                    continue
                nb, w = len(blks), BLK_W[blks[0]]
                wg_ts[name] = cpool.tile([128, NPAIR, nb, 2, w], dt.float8e4,
                                         name=f"wg_{name}")
            exp_sb = cpool.tile([128, PL, K], dt.bfloat16)
            sums_sb = cpool.tile([128, PL], dt.float32)
            zscr_sb = cpool.tile([1, 640], dt.float8e4)
            otab = cpool.tile([128, 8, 128], dt.bfloat16)
            otc = cpool.tile([128, 128], dt.bfloat16)
            otdz = cpool.tile([128, 2, 128], dt.bfloat16)

            bias = lambda lo, n: aux_sb[:, AUX_BIAS + lo:AUX_BIAS + lo + n]
            ones_ap = aux_sb[:, AUX_ONES:AUX_ONES + 128]

            # preload the activation table set holding Exp so the
            # auto-inserted per-function load (1283ns) is skipped
            nc.scalar.add_instruction(mybir.InstLoadActFuncSet(
                name=nc.get_next_instruction_name(), ins=[], outs=[],
                act_func_set_id=6))

            # --- input DMAs, all on SP/HWDGE in exact stream order; the tiny
            # aux rides Pool/SWDGE and slots into a transfer gap ---
            nc.sync.dma_start(xtwk_sb[:], xtwk_d[:])
            nc.gpsimd.dma_start(aux_sb[:], aux_d[:])
            off = 0
            for name, blks, splits in TILES:
                if name == "t0":
                    nc.sync.dma_start(
                        wg_ts[name][:],
                        wg_d[:, off:off + NT * 98]
                        .rearrange("p (t c) -> p t c", t=NT, c=98))
                    off += NT * 98
                    continue
                nb, w = len(blks), BLK_W[blks[0]]
                per_pair = nb * 2 * w
                for (p0, p1) in splits:
                    nc.sync.dma_start(
                        wg_ts[name][:, p0:p1, :, :, :],
                        wg_d[:, off + p0 * per_pair: off + p1 * per_pair]
                        .rearrange("p (a b c d) -> p a b c d",
                                   a=p1 - p0, b=nb, c=2, d=w))
                off += NPAIR * per_pair

            # --- PE warm-up: zero-input matmuls ramp the tensor engine's
            # p-state while the first DMAs are in flight ---
            nc.vector.memset(zscr_sb[:], 0.0)
            fill_ps = ppool.tile([128, 128], dt.float32, tag="ps")

            def fillers(n):
                for _ in range(n):
                    nc.tensor.matmul(fill_ps[:], zscr_sb[:, 0:128],
                                     zscr_sb[:, 128:256],
                                     start=True, stop=True,
                                     skip_group_check=True)

            fillers(4)

            # --- LSE partials for the core's own 4 partitionings:
            # logits -> exp (ACT) -> row sums (DVE) -> tiny f32 output.
            # ln + cross-core sum happen on the HOST. ---
            # The host permutes the 16 contract chunks per core so the core's
            # own 2 logits chunks sit at positions 0-1 (the contract sum of
            # the main matmul is order-agnostic; wg rows are permuted to
            # match).  The SPMD program can then use fixed chunk slices.
            psL = ppool.tile([128, PL * K], dt.float32, tag="ps")
            for tt in range(TL):
                for h in range(2):
                    pl = 2 * tt + h
                    reg = psL[:, pl * K:(pl + 1) * K]
                    nc.tensor.matmul(
                        reg,
                        xtwk_sb[h * 64:h * 64 + 64, tt, :],
                        xtwk_sb[h * 64:h * 64 + 64, NT + tt, 0:K],
                        start=True, stop=False)
                    nc.tensor.matmul(reg, ones_ap, bias(pl * K, K),
                                     start=False, stop=True)
            nc.scalar.activation(exp_sb[:], psL[:, 0:PL * K],
                                 mybir.ActivationFunctionType.Exp)

            fillers(24)

            # --- main fp8 dual-row matmul, tile by tile. Per psum bank the
            # first matmul carries start=True (zeroes the bank); the last DR
            # carries stop. The host applies bsum + (-LSE) afterwards. ---
            ps_t = {}
            for name, blks, _sp in TILES:
                nb, w = len(blks), BLK_W[blks[0]]
                ps_t[name] = ppool.tile([128, nb * 128], dt.float32, tag="ps",
                                        name=f"ps_{name}")

            def tile_drs(name, blks, splits):
                wt, ps = wg_ts[name], ps_t[name]
                if name == "t0":
                    # plain fp8 matmuls, flipped: stationary = x chunk
                    # (128 cols, Ldweights-legal), moving = 98-wide block;
                    # psum lands [batch, class]
                    for t in range(NT):
                        nc.tensor.matmul(
                            ps[:, 0:98],
                            xtwk_sb[:, t, :],
                            wt[:, t, :],
                            start=(t == 0), stop=(t == NT - 1),
                            skip_group_check=True)
                    return
                nb, w = len(blks), BLK_W[blks[0]]
                for si, (p0, p1) in enumerate(splits):
                    for pi in range(p0, p1):
                        for bi in range(nb):
                            nc.tensor.matmul(
                                ps[0:w, bi * 128:(bi + 1) * 128],
                                wt[:, pi, bi, :, :],
                                xtwk_sb[:, 2 * pi:2 * pi + 2, :],
                                start=(pi == p0 == 0 and bi == 0),
                                stop=(pi == NPAIR - 1 and bi == nb - 1),
                                perf_mode=DRI, skip_group_check=True)

            # t0 tile
            tile_drs("t0", *_tile("t0"))
            fillers(16)
            tile_drs("a", *_tile("a"))
            fillers(16)
            nc.vector.tensor_scalar_add(otab[:, 0, 0:98], ps_t["t0"][:, 0:98],
                                        0.0)
            fillers(8)
            tile_drs("b", *_tile("b"))
            nc.vector.tensor_scalar_add(otab[:, 1:5, :],
                                        ps_t["a"][:, 0:512], 0.0)
            # reduce + sums output AFTER castA in DVE/Pool program order so
            # the sums transfer's DMA-engine acquire lands behind every input
            # acquire (otherwise its 56ns slips into the middle of the input
            # stream and delays the last wg byte)
            nc.vector.tensor_reduce(sums_sb[:], exp_sb[:],
                                    axis=mybir.AxisListType.X,
                                    op=mybir.AluOpType.add)
            nc.vector.tensor_scalar_add(otab[:, 7, 0:PL], sums_sb[:], 0.0)
            fillers(8)
            tile_drs("c", *_tile("c"))
            nc.scalar.dma_start(outab_d[:, 0:4, :], otab[:, 0:4, :])
            nc.scalar.activation(otab[:, 5:7, :], ps_t["b"][:, 0:256],
                                 mybir.ActivationFunctionType.Copy)
            nc.scalar.dma_start(outab_d[:, 4:8, :], otab[:, 4:8, :])
            fillers(6)
            nc.vector.tensor_scalar_add(otc[:], ps_t["c"][:, 0:128], 0.0)
            nc.sync.dma_start(outc_d[:], otc[:])
            tile_drs("d", *_tile("d"))
            fillers(6)
            nc.scalar.activation(otdz[:, 0, :], ps_t["d"][:, 0:128],
                                 mybir.ActivationFunctionType.Copy)
            tile_drs("z", *_tile("z"))
            nc.vector.tensor_scalar_add(otdz[:, 1, :], ps_t["z"][:, 0:128],
                                        0.0)
            nc.sync.dma_start(outdz_d[:], otdz[:])

    nc.compile()
    _cached["nc"] = nc
    return nc


def _tile(name):
    for n, blks, splits in TILES:
        if n == name:
            return blks, splits
    raise KeyError(name)


def _prep_inputs(x, W, b, idx):
    """Host-side data prep -> per-core input maps."""
    x = np.asarray(x, dtype=np.float32) * XSCALE
    W = np.asarray(W, dtype=np.float32) * (1.0 / XSCALE)
    b = np.asarray(b, dtype=np.float32)
    idx = np.asarray(idx, dtype=np.int64)

    # x^T in (s_local, chunk, b) layout: (128, NT, 128)
    xt = np.ascontiguousarray(
        x.T.reshape(NT, 128, B).transpose(1, 0, 2))

    # gathered big weight matrix: Wg[(p,s), c] = W[p, idx[p,c], s],
    # chunk-major rows: (NT, 128, C)
    Wg = W[np.arange(P)[:, None], idx]            # (P, C, S)
    Wg = np.ascontiguousarray(Wg.transpose(0, 2, 1)).reshape(NT, 128, C)
    bsum_full = b[np.arange(P)[:, None], idx].sum(axis=0)   # (C,)

    aux_base = np.zeros((1, AUX_LEN), dtype=np.float32)
    aux_base[0, AUX_ONES:AUX_ONES + 128] = 1.0

    in_maps = []
    for m in range(N_CORES):
        # per-core chunk permutation: own chunks (2m, 2m+1) first, so the
        # SPMD logits path can address them at fixed positions 0-1
        perm = [2 * m, 2 * m + 1] + [t for t in range(NT)
                                     if t not in (2 * m, 2 * m + 1)]
        xtp = xt[:, perm, :].reshape(128, NT * 128)
        Wgp = Wg[perm].reshape(P * S, C)

        # per-core wk shard: local chunks tt=0,1 are global chunks 2m+tt
        wk = np.zeros((128, TL, 128), dtype=np.float32)
        for tt in range(TL):
            t = 2 * m + tt
            wk[0:64, tt, 0:K] = W[2 * t].T
            wk[64:128, tt, 0:K] = W[2 * t + 1].T
        xtwk = np.concatenate([xtp.reshape(128, NT, 128), wk],
                              axis=1).astype(F8)

        # per-tile wg shard: t0 = plain chunk-major (98 real classes, no
        # pad); the rest = dual-row interleaved 128-wide blocks
        Wcore = Wgp[:, m * CS:(m + 1) * CS]           # (2048, 1250)
        segs = []
        for name, blks, _sp in TILES:
            if name == "t0":
                t0 = Wcore[:, 0:98].reshape(NT, 128, 98).transpose(1, 0, 2)
                segs.append(np.ascontiguousarray(t0).reshape(128, -1))
                continue
            nb, w = len(blks), BLK_W[blks[0]]
            c_lo = 98 + (blks[0] - 1) * 128
            Wblk = Wcore[:, c_lo:c_lo + nb * w]
            M4 = Wblk.reshape(NPAIR, 2, 128, nb, w)   # [pi, q, j, bi, cc]
            rev = M4[:, :, :, :, ::-1]
            inter = rev.transpose(2, 0, 3, 4, 1)      # [j, pi, bi, cc_r, q]
            segs.append(np.ascontiguousarray(inter).reshape(128, -1))
        wg = np.concatenate(segs, axis=1).astype(F8)
        assert wg.shape[1] == WG_LEN, wg.shape

        aux = aux_base.copy()
        aux[0, AUX_BIAS:AUX_BIAS + PL * K] = \
            b[PL * m:PL * (m + 1)].reshape(-1)
        in_maps.append({"xtwk": xtwk, "wg": wg, "aux": aux.astype(F8),
                        "_bsum": bsum_full[m * CS:(m + 1) * CS]})
    return in_maps


def kernel(x, W, b, partitionings):
    nc = _build_program()
    in_maps = _prep_inputs(x, W, b, partitionings)
    dev_maps = [{k: v for k, v in im.items() if not k.startswith("_")}
                for im in in_maps]
    res = run_bass_kernel_spmd(nc, dev_maps, list(range(N_CORES)))

    # LSE[b] = sum over all 32 p of ln(exp-sum); each core did 4 p's,
    # shipped bf16 in otab slot 7
    sums = np.concatenate(
        [np.asarray(res.results[m]["outab"])[:, 7, 0:PL].astype(np.float32)
         for m in range(N_CORES)], axis=1)                    # (128, 32)
    lse = np.log(sums).sum(axis=1, keepdims=True)             # (128, 1)

    cores = []
    for m in range(N_CORES):
        r = res.results[m]
        blkcols = []
        ab = np.asarray(r["outab"]).astype(np.float32)        # (128, 7, 128)
        oc = np.asarray(r["outc"]).astype(np.float32)         # (128, 128)
        dz = np.asarray(r["outdz"]).astype(np.float32)        # (128, 2, 128)
        blkcols.append(ab[:, 0, 0:98])                        # block 0 (98)
        for k in range(1, 7):
            blkcols.append(ab[:, k, :].T)                     # blocks 1-6
        blkcols.append(oc.T)                                  # block 7
        blkcols.append(dz[:, 0, :].T)                         # block 8
        blkcols.append(dz[:, 1, :].T)                         # block 9
        core_out = np.concatenate(blkcols, axis=1)            # (128, 1250)
        core_out += in_maps[m]["_bsum"][None, :]
        cores.append(core_out)
    out = np.concatenate(cores, axis=1)
    return (out - lse).astype(np.float32)
